# revision 2
# baseline (speedup 1.0000x reference)
"""QRNN fo-pooling kernel for Trainium2 (8 NeuronCores, batch-sharded).

c_t = F_t * c_{t-1} + (1 - F_t) * Z_t ;  h_t = O_t * c_t
Y: [T=4096, B=16, 3H=768] fp32, init_c: [16, 256] -> (h [T,16,256], c_last [16,256])

Strategy per core (2 batches => C = 512 channels):
  - stream T in 16 super-tiles of 256 t-rows ([128 partitions, 2 blocks, 1536])
  - PE-transpose F_raw / Z_raw 128x128 blocks into PSUM
  - ScalarE applies sigmoid / sigmoid(-x) / tanh while evacuating PSUM -> SBUF
    into [channel, t] layout
  - VectorE: zf = sigmoid(-F_raw) * tanh(Z_raw), then hardware
    tensor_tensor_scan (state = f*state + zf) per 128-channel group, chained
    across super-tiles via initial=prev[:, -1:]
  - PE-transpose c back to [t, channel], multiply by O (sigmoid read applied
    on the natural-layout Y tile), DMA out
"""

import numpy as np

import concourse.bacc as bacc
import concourse.tile as tile
import concourse.mybir as mybir
from concourse.bass_utils import run_bass_kernel_spmd
from concourse.masks import make_identity

N_CORES = 8
T, B, H = 4096, 16, 256
B_LOC = B // N_CORES            # 2 batches per core
C = B_LOC * H                   # 512 channels per core
W = B_LOC * 3 * H               # 1536 input columns per t-row
NG = C // 128                   # 4 channel groups of 128
TPB = 256                       # t-rows per super-tile (2 blocks of 128)
NST = T // TPB                  # 16 super-tiles

F32 = mybir.dt.float32
AF = mybir.ActivationFunctionType
ALU = mybir.AluOpType


def _fcol(g: int) -> int:
    # channel group g = b*2 + hb covers channels [g*128, (g+1)*128)
    # (channel c = b*256 + h). F_raw column of (b, h): b*768 + h.
    b, hb = divmod(g, 2)
    return b * 768 + hb * 128


def _body(tc: tile.TileContext, Y, ic, h, cl, ctx):
    nc = tc.nc

    consts = ctx.enter_context(tc.tile_pool(name="consts", bufs=1))
    ypool = ctx.enter_context(tc.tile_pool(name="ypool", bufs=3))
    actp = ctx.enter_context(tc.tile_pool(name="actp", bufs=3))
    cpool = ctx.enter_context(tc.tile_pool(name="cpool", bufs=3))
    hpool = ctx.enter_context(tc.tile_pool(name="hpool", bufs=3))
    ps_in = ctx.enter_context(tc.tile_pool(name="ps_in", bufs=2, space="PSUM"))
    ps_out = ctx.enter_context(tc.tile_pool(name="ps_out", bufs=2, space="PSUM"))

    identity = consts.tile([128, 128], F32)
    make_identity(nc, identity)
    init_sb = consts.tile([128, NG], F32)
    nc.sync.dma_start(out=init_sb, in_=ic.rearrange("(g p) -> p g", g=NG))

    prev_ct = None
    for k in range(NST):
        r0 = k * TPB
        ysuper = ypool.tile([128, 2, W], F32)
        nc.sync.dma_start(
            out=ysuper,
            in_=Y[r0:r0 + TPB, :].rearrange("(j p) w -> p j w", j=2),
        )

        sig_f = actp.tile([128, NG, TPB], F32)
        omf = actp.tile([128, NG, TPB], F32)
        tanh_z = actp.tile([128, NG, TPB], F32)
        zf = actp.tile([128, NG, TPB], F32)

        for j in range(2):
            frawT = ps_in.tile([128, NG, 128], F32)
            zrawT = ps_in.tile([128, NG, 128], F32)
            for g in range(NG):
                fc = _fcol(g)
                nc.tensor.transpose(frawT[:, g, :], ysuper[:, j, fc:fc + 128], identity)
                nc.tensor.transpose(zrawT[:, g, :], ysuper[:, j, fc + 256:fc + 384], identity)
            js = slice(j * 128, (j + 1) * 128)
            nc.scalar.activation(sig_f[:, :, js], frawT, AF.Sigmoid)
            nc.scalar.activation(omf[:, :, js], frawT, AF.Sigmoid, scale=-1.0)
            nc.scalar.activation(tanh_z[:, :, js], zrawT, AF.Tanh)

        nc.vector.tensor_mul(zf, omf, tanh_z)

        ct = cpool.tile([128, NG, TPB], F32)
        for g in range(NG):
            nc.vector.tensor_tensor_scan(
                out=ct[:, g, :],
                data0=sig_f[:, g, :],
                data1=zf[:, g, :],
                initial=(init_sb[:, g:g + 1] if k == 0 else prev_ct[:, g, TPB - 1:TPB]),
                op0=ALU.mult,
                op1=ALU.add,
            )

        ctb = ps_out.tile([128, 2, C], F32)
        for j in range(2):
            for g in range(NG):
                nc.tensor.transpose(
                    ctb[:, j, g * 128:(g + 1) * 128],
                    ct[:, g, j * 128:(j + 1) * 128],
                    identity,
                )

        sig_o = hpool.tile([128, 2, B_LOC, H], F32)
        nc.scalar.activation(
            sig_o,
            ysuper.rearrange("p j (b x) -> p j b x", b=B_LOC)[:, :, :, 2 * H:3 * H],
            AF.Sigmoid,
        )
        h_sb = hpool.tile([128, 2, C], F32)
        nc.vector.tensor_mul(
            h_sb.rearrange("p j (b x) -> p j b x", b=B_LOC),
            ctb.rearrange("p j (b x) -> p j b x", b=B_LOC),
            sig_o,
        )
        nc.sync.dma_start(
            out=h[r0:r0 + TPB, :].rearrange("(j p) c -> p j c", j=2),
            in_=h_sb,
        )
        prev_ct = ct

    nc.sync.dma_start(
        out=cl.rearrange("(g p) -> p g", g=NG),
        in_=prev_ct[:, :, TPB - 1:TPB],
    )


_NC_CACHE = None


def _get_nc():
    global _NC_CACHE
    if _NC_CACHE is not None:
        return _NC_CACHE
    from contextlib import ExitStack

    nc = bacc.Bacc("TRN2", target_bir_lowering=False, debug=False,
                   num_devices=N_CORES)
    Y = nc.dram_tensor("Y", [T, W], F32, kind="ExternalInput").ap()
    ic = nc.dram_tensor("ic", [C], F32, kind="ExternalInput").ap()
    h = nc.dram_tensor("h", [T, C], F32, kind="ExternalOutput").ap()
    cl = nc.dram_tensor("cl", [C], F32, kind="ExternalOutput").ap()
    with tile.TileContext(nc) as tc:
        with ExitStack() as ctx:
            _body(tc, Y, ic, h, cl, ctx)
    nc.compile()
    _NC_CACHE = nc
    return _NC_CACHE


def kernel(Y, init_c, _trace=False, _trace_kwargs=None):
    Y = np.asarray(Y, dtype=np.float32)
    init_c = np.asarray(init_c, dtype=np.float32)
    nc = _get_nc()
    in_maps = []
    for i in range(N_CORES):
        ys = np.ascontiguousarray(Y[:, B_LOC * i:B_LOC * (i + 1), :]).reshape(T, W)
        ics = np.ascontiguousarray(init_c[B_LOC * i:B_LOC * (i + 1), :]).reshape(C)
        in_maps.append({"Y": ys, "ic": ics})
    res = run_bass_kernel_spmd(
        nc, in_maps, core_ids=list(range(N_CORES)),
        trace=_trace, **(_trace_kwargs or {}),
    )
    h_full = np.empty((T, B, H), np.float32)
    cl_full = np.empty((B, H), np.float32)
    for i in range(N_CORES):
        h_full[:, B_LOC * i:B_LOC * (i + 1), :] = (
            res.results[i]["h"].reshape(T, B_LOC, H))
        cl_full[B_LOC * i:B_LOC * (i + 1), :] = (
            res.results[i]["cl"].reshape(B_LOC, H))
    kernel._last_result = res
    return h_full, cl_full
